# revision 19
# baseline (speedup 1.0000x reference)
"""Differential attention kernel for 8 Trainium2 NeuronCores.

Sharding: batch x head-group. Core c handles batch b = c//4, heads
[4g, 4g+4) with g = c%4. Each core computes Q/K/V projections for its
heads over the full sequence, causal differential attention, and its
partial O-projection; the host sums the 4 partials per batch (bf16
partials, f32 accumulation).

Differential attention trick: score = (q1.k1 - lam*q2.k2) * scale is a
single K=128 matmul with stacked [q1*scale; -lam*scale*q2] and [k1; k2]
head vectors (scales folded into the projection weights on the host).

Softmax: scores are computed transposed (keys on partitions, queries
free), exp'd without max subtraction, and the denominator comes free
from a ones-column appended to V in the P@V matmul. Causality is
structural (upper blocks skipped, diagonal blocks zeroed post-exp).

Schedule: attention is emitted as per-key-tile steps (one [128,2,512]
score tile covering both heads of a pair, one exp call). Because the
attention pipeline is ACT(exp)-paced, PE "filler" work is emitted
between each step's score and P@V matmuls so the PE never drains at an
exp wait: during pair 0 the fillers are the V-projection chains (paced
just ahead of their P@V consumers) and the h2/h3 projection chains;
during pair 1 they are the O-projection tiles of completed chunks.
Input DMAs are batched into ~1-2MB pieces issued in consumption order.
"""
import math
from contextlib import ExitStack

import numpy as np
import ml_dtypes

S = 2048
H = 2048
NH = 16
HD = 64
NHC = 4          # heads per core
BF = ml_dtypes.bfloat16

_CACHED_NC = None


def _build_nc():
    import concourse.mybir as mybir
    import concourse.tile as tile
    from concourse import bacc
    from concourse.bass_types import AP

    bf16 = mybir.dt.bfloat16
    f32 = mybir.dt.float32
    AF = mybir.ActivationFunctionType

    nc = bacc.Bacc(None, target_bir_lowering=False)
    hT = nc.declare_dram_parameter("hT", [H, S], bf16, isOutput=False)
    wq = nc.declare_dram_parameter("wq", [H, NHC * 128], bf16, isOutput=False)
    wk = nc.declare_dram_parameter("wk", [H, NHC * 128], bf16, isOutput=False)
    wv = nc.declare_dram_parameter("wv", [H, NHC * 65], bf16, isOutput=False)
    wo = nc.declare_dram_parameter("wo", [128, 2, S], bf16, isOutput=False)
    out = nc.declare_dram_parameter("out", [S, H], bf16, isOutput=True)

    KT = H // 128    # 16 contraction tiles for projections
    NQ = S // 512    # 4 query chunks
    NS = S // 128    # 16 seq tiles

    def ktiled(param, k0, nk, cols):
        """[128, nk, cols] view of rows [k0*128, (k0+nk)*128) of a
        [H, cols] DRAM param (partition-major, matching an SBUF
        [128, nk, cols] destination)."""
        base = param[k0 * 128:(k0 + 1) * 128, :]
        return AP(base.tensor, k0 * 128 * cols,
                  [[cols, 128], [128 * cols, nk], [1, cols]])

    with tile.TileContext(nc) as tc:
        with ExitStack() as ctx:
            # ---- persistent SBUF ----
            sb = ctx.enter_context(tc.tile_pool(name="sb", bufs=1))
            qk_sb = ctx.enter_context(tc.tile_pool(name="qk", bufs=1))
            ht_all = sb.tile([128, KT, S], bf16)          # hidden^T
            wq_sb = sb.tile([128, KT, NHC * 128], bf16)
            wk_sb = sb.tile([128, KT, NHC * 128], bf16)
            wv_sb = sb.tile([128, KT, NHC * 65], bf16)
            wo_sb = sb.tile([128, 2, S], bf16)            # head-pair stacked Wo rows
            qT = qk_sb.tile([128, NHC, S], bf16)          # [q1*s; -lam*s*q2] per head
            kT = qk_sb.tile([128, NHC, S], bf16)          # [k1; k2] per head
            v4 = qk_sb.tile([128, NS, NHC * 65], bf16)    # V tiles + ones cols
            avt = qk_sb.tile([128, 2, S], bf16)           # attn_out^T, head pairs stacked
            # warm the ACT exp table while DMAs stream in
            warm = sb.tile([1, 16], f32)
            nc.vector.memset(warm[:], 1.0)
            nc.scalar.activation(warm[:], warm[:], AF.Exp)

            # ---- input DMAs: each HWDGE ring drains ~serially, so the
            # three DMA queues run in parallel: hT on the sync ring
            # (k-ascending, 2 k-tiles per piece), wq/wk on the ScalarE
            # ring, wv/wo on the gpsimd SWDGE queue ----
            nc.scalar.dma_start(out=wq_sb[:, 0:4, :], in_=ktiled(wq, 0, 4, NHC * 128))
            nc.scalar.dma_start(out=wk_sb[:, 0:4, :], in_=ktiled(wk, 0, 4, NHC * 128))
            for p in range(8):
                nc.sync.dma_start(out=ht_all[:, 2 * p:2 * p + 2, :],
                                  in_=ktiled(hT, 2 * p, 2, S))
            for k0 in range(4, KT, 4):
                nc.scalar.dma_start(out=wq_sb[:, k0:k0 + 4, :],
                                    in_=ktiled(wq, k0, 4, NHC * 128))
                nc.scalar.dma_start(out=wk_sb[:, k0:k0 + 4, :],
                                    in_=ktiled(wk, k0, 4, NHC * 128))
            nc.gpsimd.dma_start(out=wv_sb[:, :, :], in_=ktiled(wv, 0, KT, NHC * 65))
            nc.gpsimd.dma_start(out=wo_sb[:], in_=wo[:, :, :])

            # ---- phase A: q/k projections for heads 0,1. Warm-up matmuls
            # run while the first DMAs land so the PE HAM clock reaches
            # 2.4GHz before real work; evacuations alternate DVE/ScalarE
            # so the last chain's copies don't gate the attention PSUM ----
            wrm = sb.tile([128, 128], bf16)
            nc.vector.memset(wrm[:], 0.001)
            with tc.tile_pool(name="pjA", bufs=2, space="PSUM") as pjA:
                # parity shim: take buf0 first so the 4 chains land on
                # bufs 0,1,0,1 with the LAST chain on buf1 — then the
                # attention sc tiles (atp arena base = buf0) only wait on
                # the second-to-last chain's evacuations
                pjA.tile([128, S], f32, tag="qkp", name="shim")
                wps = pjA.tile([128, S], f32, tag="qkp", name="warm")
                for i in range(40):
                    nc.tensor.matmul(wps[:, 0:128], lhsT=wrm[:], rhs=wrm[:],
                                     start=True, stop=True)
                for h in range(2):
                    hs = slice(h * 128, (h + 1) * 128)
                    for w_sb, dst in ((wq_sb, qT), (wk_sb, kT)):
                        pp = pjA.tile([128, S], f32, tag="qkp")
                        for k in range(KT):
                            for nj in range(NQ):
                                nc.tensor.matmul(pp[:, nj * 512:(nj + 1) * 512],
                                                 lhsT=w_sb[:, k, hs],
                                                 rhs=ht_all[:, k, nj * 512:(nj + 1) * 512],
                                                 start=(k == 0), stop=(k == KT - 1))
                        for nj in range(NQ):
                            if nj % 2:
                                nc.scalar.copy(dst[:, h, nj * 512:(nj + 1) * 512],
                                               pp[:, nj * 512:(nj + 1) * 512])
                            else:
                                nc.vector.tensor_copy(dst[:, h, nj * 512:(nj + 1) * 512],
                                                      pp[:, nj * 512:(nj + 1) * 512])

            # ---- phases B/C: attention + interleaved fillers ----
            att_work = ctx.enter_context(tc.tile_pool(name="attw", bufs=3))
            nrm_work = ctx.enter_context(tc.tile_pool(name="nrmw", bufs=2))
            oout_sb = ctx.enter_context(tc.tile_pool(name="oout", bufs=4))
            atp = ctx.enter_context(tc.tile_pool(name="atp", bufs=1, space="PSUM"))

            fillq = []   # flat queue of filler closures (~2-4 PE MMs each)

            def emit_fill(n):
                for _ in range(min(n, len(fillq))):
                    fillq.pop(0)()

            def vproj_chain(st):
                """V projection for seq tile st: 4 subunits of 4 matmuls;
                last evacuates into v4 and sets the ones columns."""
                state = {}
                def sub(k0, st=st, state=state):
                    if k0 == 0:
                        state["t"] = atp.tile([128, 512], f32, tag="fill", bufs=2, name=f"vp{st}")
                    t = state["t"]
                    for k in range(k0, k0 + 4):
                        nc.tensor.matmul(t[:, 0:NHC * 65],
                                         lhsT=ht_all[:, k, st * 128:(st + 1) * 128],
                                         rhs=wv_sb[:, k, :],
                                         start=(k == 0), stop=(k == KT - 1))
                    if k0 == 12:
                        nc.vector.tensor_copy(v4[:, st, :], t[:, 0:NHC * 65])
                        for j in range(NHC):
                            nc.gpsimd.memset(v4[:, st, j * 65 + 64:j * 65 + 65], 1.0)
                return [lambda k0=k0: sub(k0) for k0 in range(0, KT, 4)]

            def proj_chain(h, w_sb, dst, nj):
                """One [128,512] q/k projection chain for head h chunk nj."""
                hs = slice(h * 128, (h + 1) * 128)
                state = {}
                def sub(k0, state=state):
                    if k0 == 0:
                        state["t"] = atp.tile([128, 512], f32, tag="fill", bufs=2,
                                              name=f"pj{h}{nj}")
                    t = state["t"]
                    for k in range(k0, k0 + 4):
                        nc.tensor.matmul(t[:], lhsT=w_sb[:, k, hs],
                                         rhs=ht_all[:, k, nj * 512:(nj + 1) * 512],
                                         start=(k == 0), stop=(k == KT - 1))
                    if k0 == 12:
                        nc.vector.tensor_copy(dst[:, h, nj * 512:(nj + 1) * 512], t[:])
                return [lambda k0=k0: sub(k0) for k0 in range(0, KT, 4)]

            def oproj_unit(qi, nch, evac_scalar=False):
                """Two subunits: the p=0 matmul, then p=1 + evacuation.
                evac_scalar routes the PSUM->SBUF cast to ScalarE (idle
                after the last exp) so the tail drain isn't DVE-paced."""
                state = {}
                def sub(p, state=state):
                    if p == 0:
                        state["t"] = atp.tile([128, 512], f32, tag="fill", bufs=2,
                                              name=f"op{qi}{nch}")
                    op = state["t"]
                    nc.tensor.matmul(op[:],
                                     lhsT=avt[:, p, qi * 128:(qi + 1) * 128],
                                     rhs=wo_sb[:, p, nch * 512:(nch + 1) * 512],
                                     start=(p == 0), stop=(p == 1))
                    if p == 1:
                        ot = oout_sb.tile([128, 512], bf16, tag="ot")
                        if evac_scalar and (qi + nch) % 2:
                            nc.scalar.copy(ot[:], op[:])
                        else:
                            nc.vector.tensor_copy(ot[:], op[:])
                        nc.sync.dma_start(
                            out=out[qi * 128:(qi + 1) * 128, nch * 512:(nch + 1) * 512],
                            in_=ot[:])
                return [lambda p=p: sub(p) for p in range(2)]

            def norm_pair(hp, nj, avs, last=False):
                """Scale av rows 0-63 by 1/denominator (av row 64) and
                write into avt. The [1,512] denominators are folded into
                a [128,8] tile over DMA so the DVE iterative-divide
                reciprocal uses all lanes. For the final norm (the only
                one on the critical path) the PSUM evacuations split
                across DVE+ScalarE and the small DMAs ride the ScalarE
                HWDGE ring so they don't queue behind output DMAs."""
                qs = slice(nj * 512, (nj + 1) * 512)
                dmae = nc.scalar if last else nc.sync
                araws = []
                dfold = nrm_work.tile([128, 8], f32, tag="dfold")
                for i in range(2):
                    den = nrm_work.tile([65, 512], f32, tag="den", bufs=4)
                    araw = nrm_work.tile([64, 512], bf16, tag="araw", bufs=4)
                    if i:
                        nc.scalar.copy(den[64:65, :], avs[i][64:65, :])
                        nc.scalar.copy(araw[:], avs[i][0:64, :])
                    else:
                        nc.vector.tensor_copy(den[64:65, :], avs[i][64:65, :])
                        nc.vector.tensor_copy(araw[:], avs[i][0:64, :])
                    dmae.dma_start(out=dfold[:, 4 * i:4 * i + 4], in_=den[64:65, :])
                    araws.append(araw)
                nc.vector.reciprocal(dfold[:], dfold[:])
                for i in range(2):
                    rc0 = nrm_work.tile([1, 512], f32, tag="rc0", bufs=2)
                    dmae.dma_start(out=rc0[:], in_=dfold[:, 4 * i:4 * i + 4])
                    bcs = nrm_work.tile([64, 512], f32, tag="bcs")
                    nc.gpsimd.partition_broadcast(bcs[:], rc0[:])
                    if i:
                        om = nrm_work.tile([64, 512], bf16, tag="om")
                        nc.vector.tensor_mul(om[:], araws[i][:], bcs[:])
                        dmae.dma_start(out=avt[64:128, hp, qs], in_=om[:])
                    else:
                        nc.vector.tensor_mul(avt[0:64, hp, qs], araws[i][:], bcs[:])

            def attention_pair(hp, nj, fill_n, last=False):
                qs = slice(nj * 512, (nj + 1) * 512)
                nblk = 4 * nj + 4
                ha, hb = 2 * hp, 2 * hp + 1
                avs = [atp.tile([65, 512], f32, tag="av", bufs=2, name=f"av{hp}{nj}{i}")
                       for i in range(2)]
                for ki in range(nblk):
                    n = fill_n(nj, ki) if callable(fill_n) else fill_n
                    sc = atp.tile([128, 2, 512], f32, tag="sc", bufs=2, name=f"sc{hp}{nj}{ki}")
                    for i, h in enumerate((ha, hb)):
                        nc.tensor.matmul(sc[:, i, :],
                                         lhsT=kT[:, h, ki * 128:(ki + 1) * 128],
                                         rhs=qT[:, h, qs], start=True, stop=True)
                    emit_fill(n)
                    pt = att_work.tile([128, 2, 512], bf16, tag="pt", bufs=4,
                                       name=f"pt{hp}{nj}{ki}")
                    nc.scalar.activation(pt[:, :, :], sc[:, :, :], AF.Exp)
                    uu = ki - 4 * nj  # >=0 on the diagonal blocks
                    if uu >= 0:
                        nc.gpsimd.affine_select(
                            out=pt[:, :, uu * 128:(uu + 1) * 128],
                            in_=pt[:, :, uu * 128:(uu + 1) * 128],
                            compare_op=mybir.AluOpType.is_ge,
                            fill=0.0,
                            base=0,
                            channel_multiplier=-1,
                            pattern=[[0, 2], [1, 128]],
                        )
                    for i, h in enumerate((ha, hb)):
                        if uu >= 0:
                            nc.tensor.matmul(avs[i][:, uu * 128:512],
                                             lhsT=v4[:, ki, h * 65:(h + 1) * 65],
                                             rhs=pt[:, i, uu * 128:512],
                                             start=(ki == 0), stop=(ki == nblk - 1))
                        else:
                            nc.tensor.matmul(avs[i][:],
                                             lhsT=v4[:, ki, h * 65:(h + 1) * 65],
                                             rhs=pt[:, i, :],
                                             start=(ki == 0), stop=(ki == nblk - 1))
                norm_pair(hp, nj, avs, last=last)

            # phase B: pair 0; fillers = V proj (st0-3 exactly paced ahead
            # of chunk 0's P@V), then V st4-15, then h2/h3 projections.
            for st in range(NS):
                fillq.extend(vproj_chain(st))
            for h in (2, 3):
                for w_sb, dst in ((wq_sb, qT), (wk_sb, kT)):
                    for nj in range(NQ):
                        fillq.extend(proj_chain(h, w_sb, dst, nj))
            b_step = [0]
            def b_fill(nj, ki):
                # chunk 0 must consume exactly one V chain per step; later
                # chunks pace the queue evenly over the remaining B steps
                b_step[0] += 1
                rem = 40 - b_step[0] + 1
                n = -(-len(fillq) // rem)
                return max(n, 4) if nj == 0 else n
            for nj in range(NQ):
                attention_pair(0, nj, fill_n=b_fill)
            emit_fill(len(fillq))   # drain: h2/h3 chains must finish before P1

            # phase C: pair 1; fillers = O-projection of completed chunks
            # (eager pacing pulls the backlog off the final drain).
            for nj in range(NQ):
                attention_pair(1, nj, fill_n=3, last=(nj == NQ - 1))
                for qi in range(4 * nj, 4 * nj + 4):
                    for nch in range(NQ):
                        fillq.extend(oproj_unit(qi, nch, evac_scalar=(nj == NQ - 1)))
            emit_fill(len(fillq))
    return nc


def _get_nc():
    global _CACHED_NC
    if _CACHED_NC is None:
        nc = _build_nc()
        if not nc.is_finalized():
            nc.finalize()
        _CACHED_NC = nc
    return _CACHED_NC


def _prep_in_maps(hidden_states, Wq, Wk, Wv, Wo, lambda_param):
    lam = math.tanh(math.log1p(math.exp(float(lambda_param))))
    scale = HD ** -0.5
    in_maps = []
    hTb = [np.ascontiguousarray(hidden_states[b].T).astype(BF) for b in range(2)]
    for core in range(8):
        b, g = divmod(core, 4)
        heads = range(NHC * g, NHC * g + NHC)
        wq_cols, wk_cols = [], []
        for h in heads:
            wq_cols.append(Wq[:, h * 64:(h + 1) * 64] * scale)
            wq_cols.append(Wq[:, (NH + h) * 64:(NH + h + 1) * 64] * (-lam * scale))
            wk_cols.append(Wk[:, h * 64:(h + 1) * 64])
            wk_cols.append(Wk[:, (NH + h) * 64:(NH + h + 1) * 64])
        wv_pad = np.zeros((H, NHC * 65), dtype=np.float32)
        for j, h in enumerate(heads):
            wv_pad[:, j * 65:j * 65 + 64] = Wv[:, h * 64:(h + 1) * 64]
        heads = list(heads)
        wo_sel = np.zeros((128, 2, S), dtype=np.float32)  # head-pair stacked rows
        for p in range(2):
            h0, h1 = heads[2 * p], heads[2 * p + 1]
            wo_sel[0:64, p] = Wo[h0 * 64:(h0 + 1) * 64, :]
            wo_sel[64:128, p] = Wo[h1 * 64:(h1 + 1) * 64, :]
        in_maps.append({
            "hT": hTb[b],
            "wq": np.concatenate(wq_cols, axis=1).astype(BF),
            "wk": np.concatenate(wk_cols, axis=1).astype(BF),
            "wv": wv_pad.astype(BF),
            "wo": np.ascontiguousarray(wo_sel).astype(BF),
        })
    return in_maps


def _mask_is_causal(attention_mask):
    m = np.asarray(attention_mask)
    if m.shape != (2, 1, S, S):
        return False
    neg = np.float32(np.finfo(np.float32).min)
    tri = np.tril(np.ones((S, S), dtype=bool))
    expect = np.where(tri, np.float32(0.0), neg)
    return all(np.array_equal(m[b, 0], expect) for b in range(m.shape[0]))


def _fallback(hidden_states, attention_mask, Wq, Wk, Wv, Wo, lambda_param):
    hs = hidden_states.astype(np.float32)
    lam = math.tanh(math.log1p(math.exp(float(lambda_param))))
    scaling = HD ** -0.5
    B = hs.shape[0]
    out = np.empty((B, S, H), dtype=np.float32)
    for b in range(B):
        q_all = (hs[b] @ Wq).reshape(S, 2 * NH, HD).transpose(1, 0, 2)
        k_all = (hs[b] @ Wk).reshape(S, 2 * NH, HD).transpose(1, 0, 2)
        v = (hs[b] @ Wv).reshape(S, NH, HD).transpose(1, 0, 2)
        acc = np.zeros((S, H), dtype=np.float32)
        for h in range(NH):
            s1 = q_all[h] @ k_all[h].T
            s2 = q_all[NH + h] @ k_all[NH + h].T
            sc = (s1 - lam * s2) * scaling + attention_mask[b, 0]
            sc -= sc.max(axis=-1, keepdims=True)
            p = np.exp(sc)
            p /= p.sum(axis=-1, keepdims=True)
            acc += (p @ v[h]) @ Wo[h * 64:(h + 1) * 64]
        out[b] = acc
    return out


def _run(inputs, trace=False):
    from concourse.bass_utils import run_bass_kernel_spmd

    hidden_states = np.asarray(inputs["hidden_states"], dtype=np.float32)
    attention_mask = np.asarray(inputs["attention_mask"], dtype=np.float32)
    Wq = np.asarray(inputs["Wq"], dtype=np.float32)
    Wk = np.asarray(inputs["Wk"], dtype=np.float32)
    Wv = np.asarray(inputs["Wv"], dtype=np.float32)
    Wo = np.asarray(inputs["Wo"], dtype=np.float32)
    lam_p = inputs["lambda_param"]

    if not _mask_is_causal(attention_mask):
        return _fallback(hidden_states, attention_mask, Wq, Wk, Wv, Wo, lam_p), None

    in_maps = _prep_in_maps(hidden_states, Wq, Wk, Wv, Wo, lam_p)
    nc = _get_nc()
    res = run_bass_kernel_spmd(nc, in_maps, list(range(8)), trace=trace)
    out = np.empty((2, S, H), dtype=np.float32)
    for b in range(2):
        acc = res.results[4 * b]["out"].astype(np.float32)
        for g in range(1, 4):
            acc = acc + res.results[4 * b + g]["out"].astype(np.float32)
        out[b] = acc
    return out, res


def kernel(**inputs):
    out, _ = _run(inputs, trace=False)
    return out
